# revision 13
# baseline (speedup 1.0000x reference)
"""Trainium2 Bass kernel for nn_BestRqLossNetwork (best-RQ masked-prediction loss).

Math (per the reference):
    logits  = context @ W_enc + b_enc                      # (N,T,K)
    targets = argmin_k ||normalize(feats @ proj) - cb_k||  # == argmax_k (feats@proj)·cb_k
                                                           #    (cb rows unit-norm, row norm > 0)
    loss    = mean over valid (t < lens[n]) of CE(logits, targets)

Distribution: data-parallel over the 8192 (n, t) positions — 1024 consecutive
tokens per core (each core's slab lies inside one sequence since T = 2*1024).
Weights (W_enc, codebook, proj) are replicated. Each core returns its local
(sum_nll, valid_count); the host sums the 16 scalars and divides.

Per-core pipeline, per 128-token tile (tokens on partitions):
  PE   : scores = fT8.T @ cbt8 — both replicated 8x along the contraction so
         all 128 PE rows are active (HAM keeps the clock at 8/8; with 16-row
         matmuls interleaved the PE sat at 4/8 = 1.2 GHz for the whole steady
         state). The 8x score scale is a power of two: argmax unaffected.
         logits = ctxT.T @ W (fp8 DoubleRow) into 1024-wide PSUM groups.
  ACT  : exp with row-sum accumulation (logsumexp without max subtraction:
         |logits| <= ~6 so exp cannot overflow). One deferred Ln at the end.
  DVE  : per-64-codeword maxes straight from PSUM (shaped tensor_reduce
         [P,16,64] -> [P,16]); argmax = max_index over the 128 segment maxes,
         gather that segment's 64 codewords (indirect DMA), recompute its 64
         scores against the per-token f vector, max_index within; target
         logit = dot(context_row, gathered W_enc.T row). No score staging to
         DRAM — saves ~18MB of DMA round-trip per core vs the two-level
         gather scheme.

Scheduling: all engines execute their streams IN ORDER; the emission order is
a software pipeline with one tile-phase of latency cover for each indirect
gather:
  phase j: [logits(j) groups interleaved with scores(j+1) megas]
           [chainB(j): segment rescore + W-row gather issue, spread over g]
           [dot(j-1) at g=6] [chainA(j+1) at the end: L1 argmax + segment
           gather issue]
A few warm-up matmuls on zeroed SBUF run at the very start so the PE's HAM
clock-gate reaches 2.4 GHz before the real work arrives.
"""

import numpy as np
import ml_dtypes

N, T, F, V, K = 4, 2048, 512, 16, 8192
NCORES = 8
TOK = (N * T) // NCORES   # tokens per core
P = 128                   # partitions / tokens per tile
NTILES = TOK // P         # 8
CC = F // P               # 4 contraction chunks of 128
MC = K // 1024            # 8 mega-chunks of 1024 classes
SEG = 64                  # codewords per argmax segment
NSEG = K // SEG           # 128 segments

_BF16 = ml_dtypes.bfloat16
_FP8 = ml_dtypes.float8_e4m3
_cache: dict = {}


def build_program(has_bias: bool):
    """Build + compile the single-core Bass program (run SPMD on 8 cores)."""
    from concourse import bacc
    import concourse.bass as bass
    import concourse.tile as tile
    import concourse.mybir as mybir

    dt = mybir.dt
    alu = mybir.AluOpType
    act = mybir.ActivationFunctionType

    nc = bacc.Bacc(
        "TRN2", target_bir_lowering=False, debug=False, num_devices=NCORES
    )

    ctxT = nc.dram_tensor("ctxT", [F, TOK], dt.float8e4, kind="ExternalInput").ap()
    ctx = nc.dram_tensor("ctx", [TOK, F], dt.bfloat16, kind="ExternalInput").ap()
    featsT = nc.dram_tensor("featsT", [F, TOK], dt.bfloat16, kind="ExternalInput").ap()
    w = nc.dram_tensor("w", [F, K], dt.float8e4, kind="ExternalInput").ap()
    wt = nc.dram_tensor("wt", [K, F], dt.bfloat16, kind="ExternalInput").ap()
    # codebook.T replicated 8x along the contraction dim (row g*16+v = cb[:,v])
    cbt8 = nc.dram_tensor("cbt8", [P, K], dt.bfloat16, kind="ExternalInput").ap()
    # proj replicated 8x along its output dim (col g*16+v = proj[:,v])
    proj8 = nc.dram_tensor("proj8", [F, P], dt.bfloat16, kind="ExternalInput").ap()
    # codebook as 128 segment rows: row s = cb[s*64:(s+1)*64, :] flattened
    cb64 = nc.dram_tensor("cb64", [NSEG, SEG * V], dt.bfloat16, kind="ExternalInput").ap()
    adjlen = nc.dram_tensor("adjlen", [P, 1], dt.float32, kind="ExternalInput").ap()
    tidx = nc.dram_tensor("tidx", [P, 1], dt.float32, kind="ExternalInput").ap()
    if has_bias:
        brow = nc.dram_tensor("brow", [1, K], dt.bfloat16, kind="ExternalInput").ap()
        bcol = nc.dram_tensor("bcol", [K, 1], dt.float32, kind="ExternalInput").ap()
    out2 = nc.dram_tensor("out2", [2, 1], dt.float32, kind="ExternalOutput").ap()

    with tile.TileContext(nc) as tc:
        with (
            tc.tile_pool(name="singles", bufs=1) as singles,
            tc.tile_pool(name="work", bufs=3) as work,
            tc.tile_pool(name="sc_ps", bufs=2, space="PSUM") as sc_ps_pool,
            tc.tile_pool(name="lg_ps", bufs=2, space="PSUM") as lg_ps_pool,
        ):
            # ---- resident SBUF tensors ----
            w_sb = singles.tile([P, CC, K], dt.float8e4)
            ctxT_sb = singles.tile([P, CC, TOK], dt.float8e4)
            featsT_sb = singles.tile([P, CC, TOK], dt.bfloat16)
            ctx_sb = singles.tile([P, NTILES, F], dt.bfloat16)
            cbt8_sb = singles.tile([P, K], dt.bfloat16)
            proj8_sb = singles.tile([P, CC, P], dt.bfloat16)
            fT8_sb = singles.tile([P, TOK], dt.bfloat16)
            f_sb = singles.tile([P, NTILES * V], dt.bfloat16)
            adjlen_sb = singles.tile([P, 1], dt.float32)
            tidx_sb = singles.tile([P, 1], dt.float32)
            ones_sb = singles.tile([P, 1], dt.float32)
            warm_sb = singles.tile([P, 512], dt.bfloat16)
            exp_scr = singles.tile([P, 1024], dt.bfloat16)
            dot_scr = singles.tile([P, F], dt.bfloat16)
            prod_scr = singles.tile([P, SEG * V], dt.bfloat16)
            nll_all = singles.tile([P, NTILES], dt.float32)
            cnt_all = singles.tile([P, NTILES], dt.float32)
            s_all = singles.tile([P, NTILES], dt.float32)
            lt_all = singles.tile([P, NTILES], dt.float32)
            logs_all = singles.tile([P, NTILES], dt.float32)
            stack2 = singles.tile([P, 2], dt.float32)
            out_sb = singles.tile([2, 1], dt.float32)

            # PE warm-up: matmuls on zeroed SBUF with no DMA dependency keep
            # the PE busy from t=0 so the HAM clock-gate opens to 2.4 GHz
            # while the input DMAs stream in.
            nc.vector.memset(warm_sb[:, :], 0.0)
            def emit_warm_mm(n=1):
                for _ in range(n):
                    wz = sc_ps_pool.tile([P, 16, SEG], dt.float32, tag="sp", name="wz")
                    nc.tensor.matmul(
                        out=wz[:, 0:8, :], lhsT=warm_sb[:, 0:P], rhs=warm_sb[:, :],
                        start=True, stop=True,
                    )

            emit_warm_mm(24)

            # Startup loads. The scalar queue is left EMPTY so the ACT engine
            # (exp is on the critical path) never burns time issuing DMA
            # descriptors. Bulk inputs are split between the sync queue
            # (featsT + W slices, need-ordered) and the gpsimd queue.
            for cc in range(CC):
                nc.sync.dma_start(out=featsT_sb[:, cc, :], in_=featsT[cc * P:(cc + 1) * P, :])
            # W in per-mega-chunk slices so the first logits matmul can start
            # after ~1 MB instead of the full 8 MB
            for g in range(MC):
                for cc in range(CC):
                    nc.sync.dma_start(
                        out=w_sb[:, cc, g * 1024:(g + 1) * 1024],
                        in_=w[cc * P:(cc + 1) * P, g * 1024:(g + 1) * 1024],
                    )
            for cc in range(CC):
                nc.gpsimd.dma_start(out=proj8_sb[:, cc, :], in_=proj8[cc * P:(cc + 1) * P, :])
            # cbt8 in K-halves so the first scores mega can start early
            nc.gpsimd.dma_start(out=cbt8_sb[:, 0:K // 2], in_=cbt8[:, 0:K // 2])
            for cc in range(CC):
                nc.gpsimd.dma_start(out=ctxT_sb[:, cc, :], in_=ctxT[cc * P:(cc + 1) * P, :])
            nc.gpsimd.dma_start(out=cbt8_sb[:, K // 2:], in_=cbt8[:, K // 2:])
            for j in range(NTILES):
                nc.gpsimd.dma_start(out=ctx_sb[:, j, :], in_=ctx[j * P:(j + 1) * P, :])
            nc.gpsimd.dma_start(out=adjlen_sb[:, :], in_=adjlen[:, :])
            nc.gpsimd.dma_start(out=tidx_sb[:, :], in_=tidx[:, :])
            nc.vector.memset(ones_sb[:, :], 1.0)

            if has_bias:
                onesrow_sb = singles.tile([1, P], dt.bfloat16)
                brow_sb = singles.tile([1, K], dt.bfloat16)
                nc.vector.memset(onesrow_sb[:, :], 1.0)
                nc.gpsimd.dma_start(out=brow_sb[:, :], in_=brow[:, :])

            # ---- fT8 = ((feats @ proj).T replicated 8x) : (128, TOK) bf16
            #      f_sb = per-token f vectors (tokens on partitions), per tile
            fT8_ps = lg_ps_pool.tile([P, TOK], dt.float32, tag="lp")
            for h in range(TOK // 512):
                for cc in range(CC):
                    nc.tensor.matmul(
                        out=fT8_ps[:, h * 512:(h + 1) * 512],
                        lhsT=proj8_sb[:, cc, :],
                        rhs=featsT_sb[:, cc, h * 512:(h + 1) * 512],
                        start=(cc == 0),
                        stop=(cc == CC - 1),
                    )
            nc.vector.tensor_copy(out=fT8_sb[:, :], in_=fT8_ps[:, :])
            f_ps = lg_ps_pool.tile([P, TOK], dt.float32, tag="lp")
            for j in range(NTILES):
                for cc in range(CC):
                    nc.tensor.matmul(
                        out=f_ps[:, j * V:(j + 1) * V],
                        lhsT=featsT_sb[:, cc, j * P:(j + 1) * P],
                        rhs=proj8_sb[:, cc, 0:V],
                        start=(cc == 0),
                        stop=(cc == CC - 1),
                    )
            nc.vector.tensor_copy(out=f_sb[:, :], in_=f_ps[:, 0:NTILES * V])

            # ---- software-pipelined main loop ----
            st = {}  # per-tile live tiles: cm/cseg/scw/s64/widx/wrow

            def emit_scores_mega(t, mc):
                """One 1024-wide scores mega-chunk: two full-contraction
                matmuls into a [P,16,64] PSUM tile, then per-64 segment maxes
                straight into this tile's cm array."""
                tsl = slice(t * P, (t + 1) * P)
                s = st.setdefault(t, {})
                if mc == 0:
                    s["cm"] = work.tile([P, NSEG], dt.float32, tag="cma",
                                        name=f"cm{t}", bufs=4)
                sp = sc_ps_pool.tile([P, 16, SEG], dt.float32, tag="sp")
                for h in range(2):
                    nc.tensor.matmul(
                        out=sp[:, h * 8:(h + 1) * 8, :],
                        lhsT=fT8_sb[:, tsl],
                        rhs=cbt8_sb[:, mc * 1024 + h * 512:mc * 1024 + (h + 1) * 512],
                        start=True,
                        stop=True,
                    )
                nc.vector.tensor_reduce(
                    out=s["cm"][:, mc * 16:(mc + 1) * 16],
                    in_=sp[:, :, :],
                    axis=mybir.AxisListType.X,
                    op=alu.max,
                )

            def emit_chainA(t):
                """L1 argmax over the 128 segment maxes + issue the segment
                codeword gather."""
                s = st[t]
                cm = s["cm"]
                m1 = work.tile([P, 1], dt.float32, tag="m1", name=f"m1_{t}")
                nc.vector.tensor_reduce(
                    out=m1[:, :], in_=cm[:, :], axis=mybir.AxisListType.X, op=alu.max
                )
                m8 = work.tile([P, 8], dt.float32, tag="m8", name=f"m8_{t}")
                nc.vector.tensor_copy(out=m8[:, :], in_=m1[:, 0:1].to_broadcast([P, 8]))
                c8 = work.tile([P, 8], dt.uint32, tag="c8", name=f"c8_{t}")
                nc.vector.max_index(c8[:, :], m8[:, :], cm[:, :])
                scw = work.tile([P, SEG * V], dt.bfloat16, tag="scw",
                                name=f"scw{t}")
                nc.gpsimd.indirect_dma_start(
                    out=scw[:, :],
                    out_offset=None,
                    in_=cb64[:, :],
                    in_offset=bass.IndirectOffsetOnAxis(
                        ap=c8[:, 0:1].bitcast(dt.int32), axis=0),
                )
                s["cseg"], s["scw"] = c8, scw

            def emit_chainB(t, part):
                """Rescore the gathered 64-codeword segment against this
                token's f vector; argmax within; issue the W_enc.T row
                gather. Split into parts so the DVE stream interleaves with
                the per-mega segment-max reduces."""
                s = st[t]
                if part == 0:
                    fb = f_sb[:, t * V:(t + 1) * V].rearrange(
                        "p (a v) -> p a v", a=1).to_broadcast([P, SEG, V])
                    nc.vector.tensor_tensor(
                        out=prod_scr[:, :].rearrange("p (s v) -> p s v", v=V),
                        in0=s["scw"][:, :].rearrange("p (s v) -> p s v", v=V),
                        in1=fb,
                        op=alu.mult,
                    )
                elif part == 1:
                    s64 = work.tile([P, SEG], dt.float32, tag="s64", name=f"s64_{t}")
                    nc.vector.tensor_reduce(
                        out=s64[:, :],
                        in_=prod_scr[:, :].rearrange("p (s v) -> p s v", v=V),
                        axis=mybir.AxisListType.X,
                        op=alu.add,
                    )
                    s["s64"] = s64
                elif part == 2:
                    m2 = work.tile([P, 1], dt.float32, tag="m2", name=f"m2_{t}")
                    nc.vector.tensor_reduce(
                        out=m2[:, :], in_=s["s64"][:, :],
                        axis=mybir.AxisListType.X, op=alu.max,
                    )
                    m28 = work.tile([P, 8], dt.float32, tag="m28", name=f"m28_{t}")
                    nc.vector.tensor_copy(
                        out=m28[:, :], in_=m2[:, 0:1].to_broadcast([P, 8]))
                    s["m28"] = m28
                elif part == 3:
                    s8 = work.tile([P, 8], dt.uint32, tag="s8", name=f"s8_{t}")
                    nc.vector.max_index(s8[:, :], s["m28"][:, :], s["s64"][:, :])
                    widx = work.tile([P, 1], dt.int32, tag="widx", name=f"widx{t}")
                    nc.vector.tensor_scalar(
                        out=widx[:, :], in0=s["cseg"][:, 0:1].bitcast(dt.int32),
                        scalar1=float(SEG), scalar2=None, op0=alu.mult,
                    )
                    nc.vector.tensor_tensor(
                        out=widx[:, :], in0=widx[:, :],
                        in1=s8[:, 0:1].bitcast(dt.int32), op=alu.add,
                    )
                    s["widx"] = widx
                elif part == 4:
                    wrow = work.tile([P, F], dt.bfloat16, tag="wrow", name=f"wrow{t}")
                    nc.gpsimd.indirect_dma_start(
                        out=wrow[:, :],
                        out_offset=None,
                        in_=wt[:, :],
                        in_offset=bass.IndirectOffsetOnAxis(ap=s["widx"][:, 0:1], axis=0),
                    )
                    s["wrow"] = wrow
                    if has_bias:
                        bg = work.tile([P, 1], dt.float32, tag="bg", name=f"bg{t}")
                        nc.gpsimd.indirect_dma_start(
                            out=bg[:, :],
                            out_offset=None,
                            in_=bcol[:, :],
                            in_offset=bass.IndirectOffsetOnAxis(ap=s["widx"][:, 0:1], axis=0),
                        )
                        s["bg"] = bg

            def emit_dot(t):
                """Target logit via dot(ctx_row, W_row) (gather landed during
                the previous tile phase)."""
                s = st[t]
                nc.vector.scalar_tensor_tensor(
                    out=dot_scr[:, :],
                    in0=ctx_sb[:, t, :],
                    scalar=1.0,
                    in1=s["wrow"][:, :],
                    op0=alu.mult,
                    op1=alu.mult,
                    accum_out=lt_all[:, t:t + 1],
                )
                if has_bias:
                    nc.vector.tensor_add(
                        lt_all[:, t:t + 1], lt_all[:, t:t + 1], s["bg"][:, :]
                    )
                del st[t]

            def emit_logits_group(j, g, sums):
                tsl = slice(j * P, (j + 1) * P)
                lp = lg_ps_pool.tile([P, 1024], dt.float32, tag="lp")
                for h in range(2):
                    hsl = slice(h * 512, (h + 1) * 512)
                    for cc2 in range(0, CC, 2):
                        nc.tensor.matmul(
                            out=lp[:, hsl],
                            lhsT=ctxT_sb[:, cc2:cc2 + 2, tsl],
                            rhs=w_sb[:, cc2:cc2 + 2, g * 1024 + h * 512:g * 1024 + (h + 1) * 512],
                            start=(cc2 == 0),
                            stop=(cc2 == CC - 2 and not has_bias),
                            perf_mode=mybir.MatmulPerfMode.DoubleRow,
                        )
                    if has_bias:
                        nc.tensor.matmul(
                            out=lp[:, hsl],
                            lhsT=onesrow_sb[:, :],
                            rhs=brow_sb[:, g * 1024 + h * 512:g * 1024 + (h + 1) * 512],
                            start=False,
                            stop=True,
                        )
                nc.scalar.activation(
                    out=exp_scr[:, :],
                    in_=lp[:, :],
                    func=act.Exp,
                    scale=1.0 / 64.0,
                    accum_out=sums[:, g:g + 1],
                )

            # Pre-phase: scores for tile 0 (no logits yet — the PE is gated
            # on the cbt8 load anyway) + its L1 argmax/gather.
            for mc in range(MC):
                emit_scores_mega(0, mc)
            emit_chainA(0)

            # Uniform phases: phase j runs logits(j) + scores(j+1) + the
            # pipelined argmax chains.
            for j in range(NTILES):
                sums = work.tile([P, MC], dt.float32, tag="sums", name=f"sums{j}")
                for g in range(MC):
                    if j + 1 < NTILES:
                        emit_scores_mega(j + 1, g)
                    if 1 <= g <= 4:
                        emit_chainB(j, g - 1)
                    elif g == 5:
                        emit_chainB(j, 4)
                    elif g == 6 and j >= 1:
                        emit_dot(j - 1)
                    emit_logits_group(j, g, sums)

                nc.vector.tensor_reduce(
                    out=s_all[:, j:j + 1], in_=sums[:, :],
                    axis=mybir.AxisListType.X, op=alu.add,
                )
                # valid mask: (tidx - adjlen) < -128*j  <=>  j*128 + tidx < len - t_off
                nc.vector.tensor_scalar(
                    out=cnt_all[:, j:j + 1],
                    in0=tidx_sb[:, :],
                    scalar1=adjlen_sb[:, 0:1],
                    scalar2=float(-(j * P)),
                    op0=alu.subtract,
                    op1=alu.is_lt,
                )
                if j + 1 < NTILES:
                    emit_chainA(j + 1)

            # ---- epilogue: last dot, one Ln for all tiles, nll assembly,
            # then the partition reduction via ones-matmul ----
            emit_dot(NTILES - 1)
            nc.scalar.activation(out=logs_all[:, :], in_=s_all[:, :], func=act.Ln)
            nc.vector.tensor_sub(nll_all[:, :], logs_all[:, :], lt_all[:, :])
            nc.vector.tensor_mul(nll_all[:, :], nll_all[:, :], cnt_all[:, :])
            nc.vector.tensor_reduce(
                out=stack2[:, 0:1], in_=nll_all[:, :], axis=mybir.AxisListType.X,
                op=alu.add,
            )
            nc.vector.tensor_reduce(
                out=stack2[:, 1:2], in_=cnt_all[:, :], axis=mybir.AxisListType.X,
                op=alu.add,
            )
            fin_ps = sc_ps_pool.tile([2, 1], dt.float32, tag="sp")
            nc.tensor.matmul(
                out=fin_ps[:, :], lhsT=stack2[:, :], rhs=ones_sb[:, :],
                start=True, stop=True,
            )
            nc.vector.tensor_copy(out=out_sb[:, :], in_=fin_ps[:, :])
            nc.sync.dma_start(out=out2[:, :], in_=out_sb[:, :])

    nc.compile()
    return nc


def _get_program(has_bias: bool):
    if has_bias not in _cache:
        _cache[has_bias] = build_program(has_bias)
    return _cache[has_bias]


def make_in_maps(feats, context, lens, proj_matrix, codebook, W_enc, b_enc,
                 has_bias):
    """Shard + lay out the full inputs into per-core input maps."""
    feats_f = np.ascontiguousarray(feats).reshape(N * T, F)
    ctx_f = np.ascontiguousarray(context).reshape(N * T, F)
    w_f8 = (W_enc * 64.0).astype(_FP8)
    wt_bf = np.ascontiguousarray(W_enc.T).astype(_BF16)
    cb_bf = codebook.astype(_BF16)
    cbt8_bf = np.ascontiguousarray(np.tile(cb_bf.T, (8, 1)))
    proj8_bf = np.ascontiguousarray(np.tile(proj_matrix, (1, 8))).astype(_BF16)
    cb64_bf = np.ascontiguousarray(cb_bf.reshape(NSEG, SEG * V))
    tidx_a = np.arange(P, dtype=np.float32).reshape(P, 1)

    in_maps = []
    for c in range(NCORES):
        sl = slice(c * TOK, (c + 1) * TOK)
        ctxs = ctx_f[sl]
        featss = feats_f[sl]
        n_idx = (c * TOK) // T
        t_off = (c * TOK) % T
        adj = np.full((P, 1), float(int(lens[n_idx]) - t_off), dtype=np.float32)
        m = {
            "ctxT": np.ascontiguousarray(ctxs.T).astype(_FP8),
            "ctx": ctxs.astype(_BF16),
            "featsT": np.ascontiguousarray(featss.T).astype(_BF16),
            "w": w_f8,
            "wt": wt_bf,
            "cbt8": cbt8_bf,
            "proj8": proj8_bf,
            "cb64": cb64_bf,
            "adjlen": adj,
            "tidx": tidx_a,
        }
        if has_bias:
            m["brow"] = np.ascontiguousarray(b_enc * 64.0).reshape(1, K).astype(_BF16)
            m["bcol"] = np.ascontiguousarray(b_enc).reshape(K, 1).astype(np.float32)
        in_maps.append(m)
    return in_maps


def kernel(feats, context, lens, proj_matrix, codebook, W_enc, b_enc,
           _want_results=False, _trace=False):
    from concourse.bass_utils import run_bass_kernel_spmd

    has_bias = bool(np.any(np.asarray(b_enc) != 0))
    nc = _get_program(has_bias)
    in_maps = make_in_maps(feats, context, lens, proj_matrix, codebook, W_enc,
                           b_enc, has_bias)
    res = run_bass_kernel_spmd(
        nc, in_maps, list(range(NCORES)), trace=_trace,
        trace_cores=list(range(NCORES)) if _trace else None,
    )
    num = sum(float(r["out2"][0, 0]) for r in res.results)
    cnt = sum(float(r["out2"][1, 0]) for r in res.results)
    loss = np.array(np.float32(num / max(cnt, 1.0)))
    if _want_results:
        return loss, res
    return loss


if __name__ == "__main__":
    import jax
    cpu = jax.devices("cpu")[0]
    import reference

    with jax.default_device(cpu):
        inputs = reference.setup_inputs()
        inputs = {k: np.asarray(v) for k, v in inputs.items()}
        expected = float(np.asarray(reference.reference(**inputs)))
    loss = float(kernel(**inputs))
    rel = abs(loss - expected) / max(abs(expected), 1e-30)
    print(f"expected {expected} got {loss} rel {rel:.3e}")
